# revision 19
# baseline (speedup 1.0000x reference)
"""CrossAttention kernel for 8 TRN2 NeuronCores (data-parallel over batch).

v2: fp8 DoubleRow matmuls + fused softmax denominator.

Per batch element b (one core each), all major matmuls fp8e4 DoubleRow
(contract 256/instr, 2x bf16 rate). Scales keep every fp8 tensor ~unit:
  xb = fp8(x)                      wq8 = fp8(64*Wq)
  q_psum = 64*q_true               q_bf = bf16(q_psum)
  wk8 = fp8(512*SCALE*Wk)          kT_bf = bf16(512*kT_true)
  sim_psum = 32768*sim_true        E = fp8(exp(sim_psum * 2^-15))
  wv8 = fp8(32*Wv); v8 = fp8(v_psum/8) = 4*v_true
  av DR stationary = [v8_h | ones64]: out rows 0-63 = 4*U, 64-127 = S
    (softmax denominator computed AND partition-replicated in the same MM)
  oc8 = fp8(4*U * recip(S)) = 4*oc_true
  out_psum = 256*out_true -> out = bf16(out_psum * 2^-8)
Residual + bias are added on host (out dtype bf16; attention-only values
are O(0.5) so bf16 rounding is negligible vs the 2e-2 gate).

Engines: TensorE matmuls; ACT exp; DVE reciprocal+normalize-mul;
GPSIMD q/out psum evacuation copies.
"""

import numpy as np
import ml_dtypes

import concourse.bass as bass
import concourse.mybir as mybir
import concourse.tile as tile
from concourse import bacc
from concourse.bass_utils import run_bass_kernel_spmd

HEADS = 8
DIM_HEAD = 64
SCALE = DIM_HEAD ** -0.5
DIM = 512          # channels of x
CTX_DIM = 768
N_CTX = 256        # context positions
HW = 4096          # 64*64 pixels
CH = 512           # i-chunk size
NCHUNK = HW // CH  # 8
B = 8              # batch == number of cores

F32 = mybir.dt.float32
BF16 = mybir.dt.bfloat16
FP8 = mybir.dt.float8e4
DR = mybir.MatmulPerfMode.DoubleRow

np_f8 = ml_dtypes.float8_e4m3


def build_bass(loop_n=1):
    nc = bacc.Bacc(
        "TRN2",
        target_bir_lowering=False,
        debug=False,
        num_devices=B,
    )

    xb_d = nc.declare_dram_parameter("xb", [DIM, HW], FP8, isOutput=False)
    ctx_d = nc.declare_dram_parameter("ctx8", [CTX_DIM, N_CTX], FP8, isOutput=False)
    wq_d = nc.declare_dram_parameter("wq8", [DIM, DIM], FP8, isOutput=False)
    wk_d = nc.declare_dram_parameter("wk8", [CTX_DIM, DIM], FP8, isOutput=False)
    wv_d = nc.declare_dram_parameter("wv8", [CTX_DIM, DIM], FP8, isOutput=False)
    wo_d = nc.declare_dram_parameter("wo8", [DIM, DIM], FP8, isOutput=False)
    out_d = nc.declare_dram_parameter("out", [DIM, HW], BF16, isOutput=True)

    xb_t = xb_d[:].rearrange("(t p) i -> p t i", p=128)    # [128, 4, 4096]
    ctx_t = ctx_d[:].rearrange("(t p) n -> p t n", p=128)  # [128, 6, 256]
    wq_t = wq_d[:].rearrange("(t p) e -> p t e", p=128)    # [128, 4, 512]
    wk_t = wk_d[:].rearrange("(t p) e -> p t e", p=128)    # [128, 6, 512]
    wv_t = wv_d[:].rearrange("(t p) e -> p t e", p=128)    # [128, 6, 512]
    wo_t = wo_d[:].rearrange("(t p) c -> p t c", p=128)    # [128, 4, 512]
    out_t = out_d[:].rearrange("(t p) i -> p t i", p=128)  # [128, 4, 4096]

    with tile.TileContext(nc) as tc:
        with (
            tc.tile_pool(name="wts", bufs=1) as wts,
            tc.tile_pool(name="xp", bufs=3) as xp,
            tc.tile_pool(name="qp", bufs=2) as qp,
            tc.tile_pool(name="ep", bufs=3) as ep,
            tc.tile_pool(name="rbp", bufs=3) as rbp,
            tc.tile_pool(name="ocp", bufs=2) as ocp,
            tc.tile_pool(name="outp", bufs=2) as outp,
            tc.tile_pool(name="psQ", bufs=1, space="PSUM") as psQ,
            tc.tile_pool(name="psO", bufs=1, space="PSUM") as psO,
            tc.tile_pool(name="psS", bufs=3, space="PSUM") as psS,
        ):
            # ---- load weights / context (all fp8) ----
            wq_sb = wts.tile([128, 4, DIM], FP8)
            nc.gpsimd.dma_start(out=wq_sb, in_=wq_t)
            wk_sb = wts.tile([128, 6, DIM], FP8)
            nc.gpsimd.dma_start(out=wk_sb, in_=wk_t)
            wv_sb = wts.tile([128, 6, DIM], FP8)
            nc.gpsimd.dma_start(out=wv_sb, in_=wv_t)
            wo_sb = wts.tile([128, 4, DIM], FP8)
            nc.gpsimd.dma_start(out=wo_sb, in_=wo_t)
            ctx_sb = wts.tile([128, 6, N_CTX], FP8)
            nc.gpsimd.dma_start(out=ctx_sb, in_=ctx_t)

            for _it in range(loop_n):
                # chunk-0 q-projection first: it has no dependency on
                # kT/v, and its ACT evacuation copies overlap the kT MMs.
                xb0 = xp.tile([128, 4, CH], FP8, tag="xb")
                nc.gpsimd.dma_start(out=xb0, in_=xb_t[:, :, bass.ts(0, CH)])
                qbf0 = qp.tile([128, 4, CH], BF16)
                for m in range(4):
                    qps = psQ.tile([128, CH], F32, tag="q")
                    for kk in range(0, 4, 2):
                        nc.tensor.matmul(
                            qps,
                            wq_sb[:, kk:kk + 2, bass.ts(m, 128)],
                            xb0[:, kk:kk + 2, :],
                            start=(kk == 0),
                            stop=(kk == 2),
                            perf_mode=DR,
                        )
                    nc.vector.tensor_copy(out=qbf0[:, m, :], in_=qps)

                # ---- kT = wk8.T @ ctx8, stored zero-padded to full 128
                # contract: kT128[:, jb, h, :] is [128, 128] with only the
                # 64 partitions of head h's dims nonzero, so every sim
                # matmul runs in full (128,128) tile mode (no row tiling,
                # no PE tile-mode switches against the DR matmuls). ----
                kT128 = wts.tile([128, 2, HEADS, 128], BF16, tag="kT")
                nc.vector.memset(kT128, 0.0)
                for m in range(4):
                    pt2 = psS.tile([128, 2, CH], F32, tag="sim")
                    pt = pt2[:, 0, :]
                    for kk in range(0, 6, 2):
                        nc.tensor.matmul(
                            pt[:, :N_CTX],
                            wk_sb[:, kk:kk + 2, bass.ts(m, 128)],
                            ctx_sb[:, kk:kk + 2, :],
                            start=(kk == 0),
                            stop=(kk == 4),
                            perf_mode=DR,
                        )
                    for jb in range(2):
                        for hh in range(2):
                            h0 = hh * 64
                            nc.scalar.copy(
                                out=kT128[h0:h0 + 64, jb, 2 * m + hh, :],
                                in_=pt[h0:h0 + 64, bass.ts(jb, 128)],
                            )

                # ---- vUz: [128, 2, 8, 128] = per head zero-padded v so a
                # head pair accumulates into ONE full-width psum bank:
                # even head stationary [4*v | 0], odd [0 | 4*v].  sA holds
                # the matching [ones|0]/[0|ones] stationaries used to build
                # the pair's replicated-denominator bank. Full-width banks
                # mean the DVE reciprocal+normalize run at 128 lanes. ----
                v8a = wts.tile([128, 2, HEADS, 128], FP8, tag="v8a")
                nc.vector.memset(v8a, 0.0)
                sA = wts.tile([128, 2, 2, 128], FP8, tag="sA")
                nc.vector.memset(sA, 0.0)
                nc.vector.memset(sA[:, :, 0, 0:64], 1.0)
                nc.vector.memset(sA[:, :, 1, 64:128], 1.0)
                for j in range(2):
                    pv = psO.tile([128, CH], F32, tag="o")
                    for kk in range(0, 6, 2):
                        nc.tensor.matmul(
                            pv,
                            ctx_sb[:, kk:kk + 2, bass.ts(j, 128)],
                            wv_sb[:, kk:kk + 2, :],
                            start=(kk == 0),
                            stop=(kk == 4),
                            perf_mode=DR,
                        )
                    for h in range(HEADS):
                        c0 = 64 if (h % 2) else 0
                        nc.scalar.activation(
                            out=v8a[:, j, h, c0:c0 + 64],
                            in_=pv[:, bass.ds(h * 64, 64)],
                            func=mybir.ActivationFunctionType.Copy,
                            scale=0.125,
                        )

                # ---- main loop: software-pipelined over pixel chunks.
                # At step c we emit: attention pairs of chunk c, the out-
                # projection of chunk c-1, and the q-projection of chunk
                # c+1 — interleaved so psum-evacuation copies never stall
                # the in-order tensor queue. ----
                def emit_qproj(c):
                    xb_sb = xp.tile([128, 4, CH], FP8, tag="xb")
                    nc.gpsimd.dma_start(out=xb_sb, in_=xb_t[:, :, bass.ts(c, CH)])
                    q_bf = qp.tile([128, 4, CH], BF16)
                    return xb_sb, q_bf

                def emit_qproj_m(xb_sb, q_bf, m):
                    qps = psQ.tile([128, CH], F32, tag="q")
                    for kk in range(0, 4, 2):
                        nc.tensor.matmul(
                            qps,
                            wq_sb[:, kk:kk + 2, bass.ts(m, 128)],
                            xb_sb[:, kk:kk + 2, :],
                            start=(kk == 0),
                            stop=(kk == 2),
                            perf_mode=DR,
                        )
                    nc.vector.tensor_copy(out=q_bf[:, m, :], in_=qps)

                def emit_outproj_m(oc8, o_sb, m):
                    ops_ = psO.tile([128, CH], F32, tag="o")
                    for kk in range(0, 4, 2):
                        nc.tensor.matmul(
                            ops_,
                            wo_sb[:, kk:kk + 2, bass.ts(m, 128)],
                            oc8[:, kk:kk + 2, :],
                            start=(kk == 0),
                            stop=(kk == 2),
                            perf_mode=DR,
                        )
                    nc.vector.tensor_scalar_mul(
                        out=o_sb[:, m, :], in0=ops_, scalar1=float(2.0 ** -8)
                    )

                def emit_pair(q_bf, oc8, p):
                        e_pair = []
                        for hh in range(2):
                            h = 2 * p + hh
                            e8 = ep.tile([128, 2, CH], FP8, tag="e")
                            sps = psS.tile([128, 2, CH], F32, tag="sim")
                            for j in range(2):
                                nc.tensor.matmul(
                                    sps[:, j, :],
                                    kT128[:, j, h, :],
                                    q_bf[:, p, :],
                                    start=True,
                                    stop=True,
                                    skip_group_check=True,
                                )
                            nc.scalar.activation(
                                out=e8,
                                in_=sps,
                                func=mybir.ActivationFunctionType.Exp,
                                scale=float(2.0 ** -15),
                            )
                            e_pair.append(e8)
                        # av2[:,0,:] = [4*U_even ; 4*U_odd] (zero-padded
                        # stationaries accumulate the two heads into one
                        # full-width bank); av2[:,1,:] = [S_even ; S_odd]
                        av2 = psS.tile([128, 2, CH], F32, tag="sim")
                        for hh in range(2):
                            nc.tensor.matmul(
                                av2[:, 0, :],
                                v8a[:, :, 2 * p + hh, :],
                                e_pair[hh],
                                start=(hh == 0),
                                stop=(hh == 1),
                                perf_mode=DR,
                            )
                            nc.tensor.matmul(
                                av2[:, 1, :],
                                sA[:, :, hh, :],
                                e_pair[hh],
                                start=(hh == 0),
                                stop=(hh == 1),
                                perf_mode=DR,
                            )
                        rb = rbp.tile([128, CH], F32, tag="rb")
                        nc.vector.reciprocal_approx_fast(
                            out=rb, in_=av2[:, 1, :]
                        )
                        nc.vector.tensor_mul(
                            out=oc8[:, p, :],
                            in0=av2[:, 0, :],
                            in1=rb,
                        )

                state = {"xb": xb0, "q": qbf0, "oc": None, "o": None}

                for c in range(NCHUNK):
                    q_cur = state["q"]
                    oc8 = ocp.tile([128, 4, CH], FP8)
                    oc_prev, o_prev = state["oc"], state["o"]
                    if c + 1 < NCHUNK:
                        xb_n, q_n = emit_qproj(c + 1)
                    else:
                        xb_n = q_n = None
                    o_sb = outp.tile([128, 4, CH], BF16)
                    for p in range(4):
                        emit_pair(q_cur, oc8, p)
                        if oc_prev is not None:
                            emit_outproj_m(oc_prev, o_prev, p)
                        if q_n is not None:
                            emit_qproj_m(xb_n, q_n, p)
                    if oc_prev is not None:
                        nc.gpsimd.dma_start(
                            out=out_t[:, :, bass.ts(c - 1, CH)], in_=o_prev
                        )
                    state = {"xb": xb_n, "q": q_n, "oc": oc8, "o": o_sb}

                # epilogue: out-projection of the last chunk
                oc_l, o_l = state["oc"], state["o"]
                for m in range(4):
                    emit_outproj_m(oc_l, o_l, m)
                nc.gpsimd.dma_start(
                    out=out_t[:, :, bass.ts(NCHUNK - 1, CH)], in_=o_l
                )

    nc.compile()
    return nc


_NC_CACHE = None


def _get_nc():
    global _NC_CACHE
    if _NC_CACHE is None:
        _NC_CACHE = build_bass()
    return _NC_CACHE


def make_in_maps(x, context, Wq, Wkv, Wout, bout):
    """Host-side prep: shard over batch, pre-transpose + fp8-quantize."""
    f = np.float32
    wq8 = np.ascontiguousarray(Wq.T * f(64.0)).astype(np_f8)
    wk8 = np.ascontiguousarray(Wkv[:512].T * f(512.0 * SCALE)).astype(np_f8)
    wv8 = np.ascontiguousarray(Wkv[512:].T * f(32.0)).astype(np_f8)
    wo8 = np.ascontiguousarray(Wout.T * f(64.0)).astype(np_f8)
    in_maps = []
    for b in range(B):
        in_maps.append({
            "xb": x[b].reshape(DIM, HW).astype(np_f8),
            "ctx8": np.ascontiguousarray(context[b].T).astype(np_f8),
            "wq8": wq8,
            "wk8": wk8,
            "wv8": wv8,
            "wo8": wo8,
        })
    return in_maps


def kernel(x, context, Wq, Wkv, Wout, bout):
    x = np.asarray(x, dtype=np.float32)
    context = np.asarray(context, dtype=np.float32)
    nc = _get_nc()
    in_maps = make_in_maps(x, context, np.asarray(Wq), np.asarray(Wkv),
                           np.asarray(Wout), np.asarray(bout))
    res = run_bass_kernel_spmd(nc, in_maps, core_ids=list(range(B)))
    out = np.stack(
        [res.results[b]["out"].astype(np.float32) for b in range(B)], axis=0
    )
    # residual + bias on host
    out += x.reshape(B, DIM, HW)
    out += np.asarray(bout, dtype=np.float32)[None, :, None]
    return out.reshape(B, DIM, 64, 64)
